# revision 20
# baseline (speedup 1.0000x reference)
"""Trainium2 Bass kernel for nn_DepthAttention.

Data-parallel over batch: B=8 images -> 8 NeuronCores, one image per core.
Each core runs the full pipeline for its [512, 32, 32] image:
  - all 1x1 convs as tiled matmuls (bf16 inputs, fp32 PSUM accumulation)
  - BN folds into per-channel scale/shift vectors applied on-chip
  - FiLM denorm + relu on DVE/ACT
  - depth-attention (dav) computed transposed so every matmul has its
    contraction dim on partitions with no on-chip transposes
  - final conv's BN folded into the weight host-side; residual add uses
    the original fp32 x so the passthrough term is exact.
"""

import os
import sys

for _p in ("/opt/trn_rl_repo",):
    if os.path.isdir(_p) and _p not in sys.path:
        sys.path.append(_p)

import numpy as np
import ml_dtypes

import concourse.bass as bass
import concourse.tile as tile
from concourse import bacc, mybir
from concourse.bass_utils import run_bass_kernel_spmd
from concourse.tile_rust import add_dep_helper

BF16 = ml_dtypes.bfloat16
F32 = np.float32
EPS = 1e-5
N_CORES = 8
P = 128

AF = mybir.ActivationFunctionType
ALU = mybir.AluOpType
dt = mybir.dt

# Filled by the last kernel() call (test harness reads these).
last_exec_time_ns = None
last_results = None

_compiled_nc = None


def _build(dbg=False):
    """Emit the single-core Bass/Tile program (SPMD across 8 cores)."""
    nc = bacc.Bacc(
        "TRN2", target_bir_lowering=False, debug=False, num_devices=N_CORES
    )

    def dram_in(name, shape, dtype):
        return nc.dram_tensor(name, shape, dtype, kind="ExternalInput").ap()

    # DRAM views are pre-rearranged so partitions index contiguous rows.
    x_d = dram_in("x", [512, 32, 32], dt.float32).rearrange(
        "(ko p) h w -> p ko (h w)", p=P
    )
    xbf_d = dram_in("xbf16", [512, 1024], dt.bfloat16).rearrange(
        "(ko p) n -> p ko n", p=P
    )
    wg1_d = dram_in("wg1t", [512, 1024], dt.bfloat16).rearrange(
        "(ko p) c -> p ko c", p=P
    )
    wb1_d = dram_in("wb1t", [512, 1024], dt.bfloat16).rearrange(
        "(ko p) c -> p ko c", p=P
    )
    wg1g_d = dram_in("wg1gt", [1024, 1024], dt.bfloat16).rearrange(
        "(ko p) c -> p ko c", p=P
    )
    wg1be_d = dram_in("wg1bet", [1024, 1024], dt.bfloat16).rearrange(
        "(ko p) c -> p ko c", p=P
    )
    wb1g_d = dram_in("wb1gt", [1024, 1024], dt.bfloat16).rearrange(
        "(ko p) c -> p ko c", p=P
    )
    wb1be_d = dram_in("wb1bet", [1024, 1024], dt.bfloat16).rearrange(
        "(ko p) c -> p ko c", p=P
    )
    wg2_d = dram_in("wg2t", [1024, 1024], dt.bfloat16).rearrange(
        "(ko p) c -> p ko c", p=P
    )
    wb2_d = dram_in("wb2t", [1024, 1024], dt.bfloat16).rearrange(
        "(ko p) c -> p ko c", p=P
    )
    wo_d = dram_in("wot", [512, 256], dt.bfloat16).rearrange(
        "(ko p) c -> p ko c", p=P
    )
    wd4_d = dram_in("wd4t", [256, 512], dt.bfloat16).rearrange(
        "(ko p) c -> p ko c", p=P
    )
    bo2_d = dram_in("bo2", [2, 256], dt.bfloat16)
    vec_d = dram_in("vecs", [128, 100], dt.float32)
    dbg_d = {}
    if dbg:
        for nm, shp in [
            ("green1", [128, 8, 1024]), ("blue1", [128, 8, 1024]),
            ("gdr", [128, 8, 1024]), ("bdr", [128, 8, 1024]),
            ("ot", [128, 8, 256]), ("green2", [128, 8, 1024]),
            ("blue2", [128, 8, 1024]), ("d3sb", [128, 2, 1024]),
        ]:
            dbg_d[nm] = nc.dram_tensor(
                f"dbg_{nm}", shp, dt.bfloat16, kind="ExternalOutput"
            ).ap()
        dbg_d["d2"] = nc.dram_tensor(
            "dbg_d2", [8, 128, 1024], dt.bfloat16, kind="ExternalOutput"
        ).ap()
    out_d = nc.dram_tensor(
        "out", [512, 1024], dt.float32, kind="ExternalOutput"
    ).ap().rearrange("(o p) n -> p o n", p=P)

    # vec column layout (see _prep_inputs): 12 vectors x 8 m-cols, then b4'.
    V_BG1, V_SG1, V_TG1, V_BB1, V_SB1, V_TB1 = 0, 1, 2, 3, 4, 5
    V_BG1G, V_BG1BE, V_BB1G, V_BB1BE, V_BG2, V_BB2 = 6, 7, 8, 9, 10, 11

    with tile.TileContext(nc) as tc:
        from contextlib import ExitStack

        with ExitStack() as ctx:
            wpool = ctx.enter_context(tc.tile_pool(name="wpool", bufs=4))
            w2 = ctx.enter_context(tc.tile_pool(name="w2", bufs=2))
            acts = ctx.enter_context(tc.tile_pool(name="acts", bufs=4))
            gbp = ctx.enter_context(tc.tile_pool(name="gbp", bufs=2))
            small = ctx.enter_context(tc.tile_pool(name="small", bufs=1))
            ctmp = ctx.enter_context(tc.tile_pool(name="ctmp", bufs=1))
            d2p = ctx.enter_context(tc.tile_pool(name="d2p", bufs=3))
            outp = ctx.enter_context(tc.tile_pool(name="outp", bufs=2))
            ps = ctx.enter_context(tc.tile_pool(name="ps", bufs=4, space="PSUM"))
            ps3 = ctx.enter_context(tc.tile_pool(name="ps3", bufs=1, space="PSUM"))

            def vcol(v, m):
                return vec_d  # placeholder, replaced below

            # ---- constant/small tiles ----
            vec_t = small.tile([128, 100], dt.float32, name="vec_t", tag="vec")
            nc.sync.dma_start(vec_t[:], vec_d[:])
            bo2_t = small.tile([2, 256], dt.bfloat16, name="bo2_t", tag="bo2")
            nc.sync.dma_start(bo2_t[:], bo2_d[:])
            ones2 = small.tile([2, 512], dt.bfloat16, name="ones2", tag="ones")
            nc.vector.memset(ones2[:], 1.0)

            def vcol(v, m):
                i = v * 8 + m
                return vec_t[:, i : i + 1]

            # ---- x load (bf16 copy cast on host) ----
            xbf = small.tile([128, 4, 1024], dt.bfloat16, name="xbf", tag="xbf")
            ph0 = []

            # ---- weight loads (stage A + B + prefetch) ----
            def defer(dma_inst, after, why):
                for up in after:
                    add_dep_helper(dma_inst.ins, up.ins, reason=why)
                return dma_inst

            for k in range(4):
                ph0.append(nc.sync.dma_start(xbf[:, k, :], xbf_d[:, k, :]))
            wg1 = wpool.tile([128, 4, 1024], dt.bfloat16, name="wg1", tag="w")
            for k in range(4):
                ph0.append(nc.sync.dma_start(wg1[:, k, :], wg1_d[:, k, :]))
            wb1 = wpool.tile([128, 4, 1024], dt.bfloat16, name="wb1", tag="w")
            ph0c = [defer(nc.sync.dma_start(wb1[:], wb1_d[:]), ph0, "ph0b")]
            wo = small.tile([128, 4, 256], dt.bfloat16, name="wo", tag="wo")
            ph0c.append(defer(nc.sync.dma_start(wo[:], wo_d[:]), ph0, "ph0b"))

            wg1g = wpool.tile([128, 8, 1024], dt.bfloat16, name="wg1g", tag="w")
            ph1 = [defer(nc.sync.dma_start(wg1g[:], wg1g_d[:]), ph0c, "ph1")]
            wg1be = wpool.tile([128, 8, 1024], dt.bfloat16, name="wg1be", tag="w")
            ph1.append(defer(nc.sync.dma_start(wg1be[:], wg1be_d[:]), ph0c, "ph1"))

            def conv_mm(pt, wt, m, nh, kchunks, rhs, nfree=512):
                for k in range(kchunks):
                    nc.tensor.matmul(
                        pt[:, :nfree],
                        lhsT=wt[:, k, m * 128 : (m + 1) * 128],
                        rhs=rhs[:, k, nh * nfree : (nh + 1) * nfree],
                        start=(k == 0),
                        stop=(k == kchunks - 1),
                    )

            # ---- stage A: green_1 / blue_1 ----
            green1 = acts.tile([128, 8, 1024], dt.bfloat16, name="green1", tag="a")
            for m in range(8):
                for nh in range(2):
                    pt = ps.tile([128, 512], dt.float32, name="pA", tag="ps")
                    conv_mm(pt, wg1, m, nh, 4, xbf)
                    nc.scalar.activation(
                        green1[:, m, nh * 512 : (nh + 1) * 512],
                        pt[:],
                        AF.Identity,
                        bias=vcol(V_BG1, m),
                    )
            blue1 = acts.tile([128, 8, 1024], dt.bfloat16, name="blue1", tag="a")
            for m in range(8):
                for nh in range(2):
                    pt = ps.tile([128, 512], dt.float32, name="pA2", tag="ps")
                    conv_mm(pt, wb1, m, nh, 4, xbf)
                    nc.scalar.activation(
                        blue1[:, m, nh * 512 : (nh + 1) * 512],
                        pt[:],
                        AF.Identity,
                        bias=vcol(V_BB1, m),
                    )

            # late weight loads reuse wg1/wb1 slots once stage A is done
            wb1g = wpool.tile([128, 8, 1024], dt.bfloat16, name="wb1g", tag="w")
            ph2 = [defer(nc.sync.dma_start(wb1g[:], wb1g_d[:]), ph1, "ph2")]
            wb1be = wpool.tile([128, 8, 1024], dt.bfloat16, name="wb1be", tag="w")
            ph2.append(defer(nc.sync.dma_start(wb1be[:], wb1be_d[:]), ph1, "ph2"))
            wg2 = w2.tile([128, 8, 1024], dt.bfloat16, name="wg2", tag="w2")
            defer(nc.sync.dma_start(wg2[:], wg2_d[:]), ph2, "ph3")
            wb2 = w2.tile([128, 8, 1024], dt.bfloat16, name="wb2", tag="w2")
            defer(nc.sync.dma_start(wb2[:], wb2_d[:]), ph2, "ph3")

            # ---- stage B+C interleaved per m-tile ----
            gdr = acts.tile([128, 8, 1024], dt.bfloat16, name="gdr", tag="a")
            bdr = acts.tile([128, 8, 1024], dt.bfloat16, name="bdr", tag="a")
            for m in range(8):
                g1g_t = gbp.tile([128, 1024], dt.bfloat16, name="g1g_t", tag="g1g")
                for nh in range(2):
                    pt = ps.tile([128, 512], dt.float32, name="pB", tag="ps")
                    conv_mm(pt, wg1g, m, nh, 8, green1)
                    nc.scalar.activation(
                        g1g_t[:, nh * 512 : (nh + 1) * 512],
                        pt[:],
                        AF.Identity,
                        bias=vcol(V_BG1G, m),
                    )
                g1be_t = gbp.tile([128, 1024], dt.bfloat16, name="g1be_t", tag="g1be")
                for nh in range(2):
                    pt = ps.tile([128, 512], dt.float32, name="pB2", tag="ps")
                    conv_mm(pt, wg1be, m, nh, 8, green1)
                    nc.scalar.activation(
                        g1be_t[:, nh * 512 : (nh + 1) * 512],
                        pt[:],
                        AF.Identity,
                        bias=vcol(V_BG1BE, m),
                    )
                b1g_t = gbp.tile([128, 1024], dt.bfloat16, name="b1g_t", tag="b1g")
                for nh in range(2):
                    pt = ps.tile([128, 512], dt.float32, name="pB3", tag="ps")
                    conv_mm(pt, wb1g, m, nh, 8, blue1)
                    nc.vector.tensor_scalar_add(
                        b1g_t[:, nh * 512 : (nh + 1) * 512], pt[:], vcol(V_BB1G, m)
                    )
                b1be_t = gbp.tile([128, 1024], dt.bfloat16, name="b1be_t", tag="b1be")
                for nh in range(2):
                    pt = ps.tile([128, 512], dt.float32, name="pB4", tag="ps")
                    conv_mm(pt, wb1be, m, nh, 8, blue1)
                    nc.vector.tensor_scalar_add(
                        b1be_t[:, nh * 512 : (nh + 1) * 512], pt[:], vcol(V_BB1BE, m)
                    )

                # C green: relu((s_g1*green1 + t_g1)*b1_gamma + b1_beta)
                cu = ctmp.tile([128, 1024], dt.bfloat16, name="cu", tag="cu")
                nc.vector.scalar_tensor_tensor(
                    cu[:], b1g_t[:], vcol(V_TG1, m), b1be_t[:], ALU.mult, ALU.add
                )
                cv = ctmp.tile([128, 1024], dt.bfloat16, name="cv", tag="cv")
                nc.vector.scalar_tensor_tensor(
                    cv[:], green1[:, m, :], vcol(V_SG1, m), b1g_t[:],
                    ALU.mult, ALU.mult,
                )
                cw = ctmp.tile([128, 1024], dt.bfloat16, name="cw", tag="cw")
                nc.vector.tensor_tensor(cw[:], cu[:], cv[:], ALU.add)
                nc.scalar.activation(gdr[:, m, :], cw[:], AF.Relu)

                # C blue: relu((s_b1*blue1 + t_b1)*g1_gamma + g1_beta)
                cu2 = ctmp.tile([128, 1024], dt.bfloat16, name="cu2", tag="cu")
                nc.vector.scalar_tensor_tensor(
                    cu2[:], g1g_t[:], vcol(V_TB1, m), g1be_t[:], ALU.mult, ALU.add
                )
                cv2 = ctmp.tile([128, 1024], dt.bfloat16, name="cv2", tag="cv")
                nc.vector.scalar_tensor_tensor(
                    cv2[:], blue1[:, m, :], vcol(V_SB1, m), g1g_t[:],
                    ALU.mult, ALU.mult,
                )
                cw2 = ctmp.tile([128, 1024], dt.bfloat16, name="cw2", tag="cw")
                nc.vector.tensor_tensor(cw2[:], cu2[:], cv2[:], ALU.add)
                nc.scalar.activation(bdr[:, m, :], cw2[:], AF.Relu)

            if dbg:
                nc.gpsimd.dma_start(dbg_d["green1"][:], green1[:])
                nc.gpsimd.dma_start(dbg_d["blue1"][:], blue1[:])
                nc.gpsimd.dma_start(dbg_d["gdr"][:], gdr[:])
                nc.gpsimd.dma_start(dbg_d["bdr"][:], bdr[:])

            # ---- stage E: orangeT = x^T @ WoT (fills the C->D pipeline gap) ----
            ot = acts.tile([128, 8, 256], dt.bfloat16, name="ot", tag="a")
            for nt in range(8):
                pt = ps.tile([128, 512], dt.float32, name="pE", tag="ps")
                for k in range(4):
                    nc.tensor.matmul(
                        pt[:, :256],
                        lhsT=xbf[:, k, nt * 128 : (nt + 1) * 128],
                        rhs=wo[:, k, :],
                        start=(k == 0),
                        stop=False,
                    )
                # orange bias, exact to fp32 via hi/lo rank-1 rows
                nc.tensor.matmul(
                    pt[:, :256],
                    lhsT=ones2[:, :128],
                    rhs=bo2_t[:],
                    start=False,
                    stop=True,
                )
                nc.scalar.copy(ot[:, nt, :], pt[:, :256])

            # ---- stage D: green2 / blue2 ----
            green2 = wpool.tile([128, 8, 1024], dt.bfloat16, name="green2", tag="w")
            for m in range(8):
                for nh in range(2):
                    pt = ps.tile([128, 512], dt.float32, name="pD", tag="ps")
                    conv_mm(pt, wg2, m, nh, 8, gdr)
                    nc.scalar.activation(
                        green2[:, m, nh * 512 : (nh + 1) * 512],
                        pt[:],
                        AF.Identity,
                        bias=vcol(V_BG2, m),
                    )
            blue2 = wpool.tile([128, 8, 1024], dt.bfloat16, name="blue2", tag="w")
            for m in range(8):
                for nh in range(2):
                    pt = ps.tile([128, 512], dt.float32, name="pD2", tag="ps")
                    conv_mm(pt, wb2, m, nh, 8, bdr)
                    nc.scalar.activation(
                        blue2[:, m, nh * 512 : (nh + 1) * 512],
                        pt[:],
                        AF.Identity,
                        bias=vcol(V_BB2, m),
                    )
            wd4 = w2.tile([128, 2, 512], dt.bfloat16, name="wd4", tag="w2")
            nc.sync.dma_start(wd4[:], wd4_d[:])

            if dbg:
                nc.gpsimd.dma_start(dbg_d["ot"][:], ot[:])
                nc.gpsimd.dma_start(dbg_d["green2"][:], green2[:])
                nc.gpsimd.dma_start(dbg_d["blue2"][:], blue2[:])

            # ---- stage F: davT = sigmoid(blue2^T @ green2); dav3 = of @ dav2^T ----
            d3ps = [
                ps3.tile([128, 512], dt.float32, name=f"d3ps{i}", tag=f"d3{i}")
                for i in range(4)
            ]
            def dav3_acc(m, d2):
                for ch in range(2):
                    for nh in range(2):
                        nc.tensor.matmul(
                            d3ps[ch * 2 + nh][:],
                            lhsT=ot[:, m, ch * 128 : (ch + 1) * 128],
                            rhs=d2[:, nh * 512 : (nh + 1) * 512],
                            start=(m == 0),
                            stop=(m == 7),
                        )

            prev_d2 = None
            for m in range(8):
                d2 = d2p.tile([128, 1024], dt.bfloat16, name="d2", tag="d2")
                for nh in range(2):
                    pt = ps.tile([128, 512], dt.float32, name="pF", tag="ps")
                    for k in range(8):
                        nc.tensor.matmul(
                            pt[:],
                            lhsT=blue2[:, k, m * 128 : (m + 1) * 128],
                            rhs=green2[:, k, nh * 512 : (nh + 1) * 512],
                            start=(k == 0),
                            stop=(k == 7),
                        )
                    nc.scalar.activation(
                        d2[:, nh * 512 : (nh + 1) * 512], pt[:], AF.Sigmoid
                    )
                if dbg:
                    nc.gpsimd.dma_start(dbg_d["d2"][m], d2[:])
                # dav3 for m-1 lands here so m's sigmoid latency hides
                # behind it (and m=7's sigmoid behind dav3[6])
                if prev_d2 is not None:
                    dav3_acc(m - 1, prev_d2)
                prev_d2 = d2
            dav3_acc(7, prev_d2)
            d3sb = small.tile([128, 2, 1024], dt.bfloat16, name="d3sb", tag="xbf")
            for ch in range(2):
                nc.vector.tensor_copy(d3sb[:, ch, 0:512], d3ps[ch * 2 + 0][:])
                nc.scalar.copy(d3sb[:, ch, 512:1024], d3ps[ch * 2 + 1][:])

            if dbg:
                nc.gpsimd.dma_start(dbg_d["d3sb"][:], d3sb[:])

            # ---- stage G: dav4 + residual ----
            xf32 = acts.tile([128, 4, 1024], dt.float32, name="xf32", tag="a")
            nc.sync.dma_start(xf32[:], x_d[:])
            for o in range(4):
                for nh in range(2):
                    pt = ps.tile([128, 512], dt.float32, name="pG", tag="ps")
                    for c2 in range(2):
                        nc.tensor.matmul(
                            pt[:],
                            lhsT=wd4[:, c2, o * 128 : (o + 1) * 128],
                            rhs=d3sb[:, c2, nh * 512 : (nh + 1) * 512],
                            start=(c2 == 0),
                            stop=(c2 == 1),
                        )
                    outt = outp.tile([128, 512], dt.float32, name="outt", tag="out")
                    nc.vector.scalar_tensor_tensor(
                        outt[:],
                        pt[:],
                        vec_t[:, 96 + o : 97 + o],
                        xf32[:, o, nh * 512 : (nh + 1) * 512],
                        ALU.add,
                        ALU.add,
                    )
                    nc.gpsimd.dma_start(
                        out_d[:, o, nh * 512 : (nh + 1) * 512], outt[:]
                    )

    nc.compile()
    return nc


def _prep_inputs(x, params):
    """Host-side: transpose/cast weights, fold BN, pack per-channel vectors."""
    g = lambda k: np.asarray(params[k], dtype=F32)

    def bnfold(bn):
        w = np.asarray(bn["weight"], F32)
        b = np.asarray(bn["bias"], F32)
        mu = np.asarray(bn["mean"], F32)
        var = np.asarray(bn["var"], F32)
        s = w / np.sqrt(var + EPS)
        t = b - mu * s
        return s, t

    s_g1, t_g1 = bnfold(params["g1_bn"])
    s_b1, t_b1 = bnfold(params["b1_bn"])
    s_d4, t_d4 = bnfold(params["d4_bn"])

    wT = lambda k: np.ascontiguousarray(g(k).T).astype(BF16)
    ins = {
        "wg1t": wT("g1_w"),
        "wb1t": wT("b1_w"),
        "wg1gt": wT("g1g_w"),
        "wg1bet": wT("g1be_w"),
        "wb1gt": wT("b1g_w"),
        "wb1bet": wT("b1be_w"),
        "wg2t": wT("g2_w"),
        "wb2t": wT("b2_w"),
        "wot": wT("orange_w"),
        "wd4t": np.ascontiguousarray((g("d4_w") * s_d4[:, None]).T).astype(BF16),
    }

    bo = g("orange_b")
    bo_hi = bo.astype(BF16)
    bo_lo = (bo - bo_hi.astype(F32)).astype(BF16)
    ins["bo2"] = np.ascontiguousarray(np.stack([bo_hi, bo_lo]))

    vecs = np.zeros((128, 100), dtype=F32)
    packs = [
        g("g1_b"), s_g1, t_g1, g("b1_b"), s_b1, t_b1,
        g("g1g_b"), g("g1be_b"), g("b1g_b"), g("b1be_b"),
        g("g2_b"), g("b2_b"),
    ]
    for v, vec in enumerate(packs):
        vecs[:, v * 8 : (v + 1) * 8] = vec.reshape(8, 128).T
    b4p = g("d4_b") * s_d4 + t_d4
    vecs[:, 96:100] = b4p.reshape(4, 128).T
    ins["vecs"] = vecs

    x = np.asarray(x, dtype=F32)
    in_maps = []
    for i in range(N_CORES):
        m = dict(ins)
        m["x"] = np.ascontiguousarray(x[i])
        m["xbf16"] = np.ascontiguousarray(x[i].reshape(512, 1024).astype(BF16))
        in_maps.append(m)
    return in_maps


def kernel(x, params):
    global _compiled_nc, last_exec_time_ns, last_results
    if _compiled_nc is None:
        _compiled_nc = _build()
    nc = _compiled_nc
    in_maps = _prep_inputs(x, params)
    trace = os.environ.get("BASS_KERNEL_TRACE", "0") == "1"
    res = run_bass_kernel_spmd(nc, in_maps, list(range(N_CORES)), trace=trace)
    last_exec_time_ns = res.exec_time_ns
    last_results = res
    out = np.stack(
        [res.results[i]["out"].reshape(512, 32, 32) for i in range(N_CORES)]
    )
    return out.astype(np.float32)


# revision 21
# speedup vs baseline: 1.0105x; 1.0105x over previous
"""Trainium2 Bass kernel for nn_DepthAttention.

Data-parallel over batch: B=8 images -> 8 NeuronCores, one image per core.
Each core runs the full pipeline for its [512, 32, 32] image:
  - all 1x1 convs as tiled matmuls (bf16 inputs, fp32 PSUM accumulation)
  - BN folds into per-channel scale/shift vectors applied on-chip
  - FiLM denorm + relu on DVE/ACT
  - depth-attention (dav) computed transposed so every matmul has its
    contraction dim on partitions with no on-chip transposes
  - final conv's BN folded into the weight host-side; residual add uses
    the original fp32 x so the passthrough term is exact.
"""

import os
import sys

for _p in ("/opt/trn_rl_repo",):
    if os.path.isdir(_p) and _p not in sys.path:
        sys.path.append(_p)

import numpy as np
import ml_dtypes

import concourse.bass as bass
import concourse.tile as tile
from concourse import bacc, mybir
from concourse.bass_utils import run_bass_kernel_spmd
from concourse.tile_rust import add_dep_helper

BF16 = ml_dtypes.bfloat16
F32 = np.float32
EPS = 1e-5
N_CORES = 8
P = 128

AF = mybir.ActivationFunctionType
ALU = mybir.AluOpType
dt = mybir.dt

# Filled by the last kernel() call (test harness reads these).
last_exec_time_ns = None
last_results = None

_compiled_nc = None


def _build(dbg=False):
    """Emit the single-core Bass/Tile program (SPMD across 8 cores)."""
    nc = bacc.Bacc(
        "TRN2", target_bir_lowering=False, debug=False, num_devices=N_CORES
    )

    def dram_in(name, shape, dtype):
        return nc.dram_tensor(name, shape, dtype, kind="ExternalInput").ap()

    # DRAM views are pre-rearranged so partitions index contiguous rows.
    x_d = dram_in("x", [512, 32, 32], dt.float32).rearrange(
        "(ko p) h w -> p ko (h w)", p=P
    )
    xbf_d = dram_in("xbf16", [512, 1024], dt.bfloat16).rearrange(
        "(ko p) n -> p ko n", p=P
    )
    wg1_d = dram_in("wg1t", [512, 1024], dt.bfloat16).rearrange(
        "(ko p) c -> p ko c", p=P
    )
    wb1_d = dram_in("wb1t", [512, 1024], dt.bfloat16).rearrange(
        "(ko p) c -> p ko c", p=P
    )
    wg1g_d = dram_in("wg1gt", [1024, 1024], dt.bfloat16).rearrange(
        "(ko p) c -> p ko c", p=P
    )
    wg1be_d = dram_in("wg1bet", [1024, 1024], dt.bfloat16).rearrange(
        "(ko p) c -> p ko c", p=P
    )
    wb1g_d = dram_in("wb1gt", [1024, 1024], dt.bfloat16).rearrange(
        "(ko p) c -> p ko c", p=P
    )
    wb1be_d = dram_in("wb1bet", [1024, 1024], dt.bfloat16).rearrange(
        "(ko p) c -> p ko c", p=P
    )
    wg2_d = dram_in("wg2t", [1024, 1024], dt.bfloat16).rearrange(
        "(ko p) c -> p ko c", p=P
    )
    wb2_d = dram_in("wb2t", [1024, 1024], dt.bfloat16).rearrange(
        "(ko p) c -> p ko c", p=P
    )
    wo_d = dram_in("wot", [512, 256], dt.bfloat16).rearrange(
        "(ko p) c -> p ko c", p=P
    )
    wd4_d = dram_in("wd4t", [256, 512], dt.bfloat16).rearrange(
        "(ko p) c -> p ko c", p=P
    )
    bo2_d = dram_in("bo2", [2, 256], dt.bfloat16)
    vec_d = dram_in("vecs", [128, 100], dt.float32)
    dbg_d = {}
    if dbg:
        for nm, shp in [
            ("green1", [128, 8, 1024]), ("blue1", [128, 8, 1024]),
            ("gdr", [128, 8, 1024]), ("bdr", [128, 8, 1024]),
            ("ot", [128, 8, 256]), ("green2", [128, 8, 1024]),
            ("blue2", [128, 8, 1024]), ("d3sb", [128, 2, 1024]),
        ]:
            dbg_d[nm] = nc.dram_tensor(
                f"dbg_{nm}", shp, dt.bfloat16, kind="ExternalOutput"
            ).ap()
        dbg_d["d2"] = nc.dram_tensor(
            "dbg_d2", [8, 128, 1024], dt.bfloat16, kind="ExternalOutput"
        ).ap()
    out_d = nc.dram_tensor(
        "out", [512, 1024], dt.float32, kind="ExternalOutput"
    ).ap().rearrange("(o p) n -> p o n", p=P)

    # vec column layout (see _prep_inputs): 12 vectors x 8 m-cols, then b4'.
    V_BG1, V_SG1, V_TG1, V_BB1, V_SB1, V_TB1 = 0, 1, 2, 3, 4, 5
    V_BG1G, V_BG1BE, V_BB1G, V_BB1BE, V_BG2, V_BB2 = 6, 7, 8, 9, 10, 11

    with tile.TileContext(nc) as tc:
        from contextlib import ExitStack

        with ExitStack() as ctx:
            wpool = ctx.enter_context(tc.tile_pool(name="wpool", bufs=4))
            w2 = ctx.enter_context(tc.tile_pool(name="w2", bufs=2))
            acts = ctx.enter_context(tc.tile_pool(name="acts", bufs=4))
            gbp = ctx.enter_context(tc.tile_pool(name="gbp", bufs=2))
            small = ctx.enter_context(tc.tile_pool(name="small", bufs=1))
            ctmp = ctx.enter_context(tc.tile_pool(name="ctmp", bufs=1))
            d2p = ctx.enter_context(tc.tile_pool(name="d2p", bufs=2))
            outp = ctx.enter_context(tc.tile_pool(name="outp", bufs=2))
            ps = ctx.enter_context(tc.tile_pool(name="ps", bufs=4, space="PSUM"))
            ps3 = ctx.enter_context(tc.tile_pool(name="ps3", bufs=1, space="PSUM"))

            def vcol(v, m):
                return vec_d  # placeholder, replaced below

            # ---- constant/small tiles ----
            vec_t = small.tile([128, 100], dt.float32, name="vec_t", tag="vec")
            nc.sync.dma_start(vec_t[:], vec_d[:])
            bo2_t = small.tile([2, 256], dt.bfloat16, name="bo2_t", tag="bo2")
            nc.sync.dma_start(bo2_t[:], bo2_d[:])
            ones2 = small.tile([2, 512], dt.bfloat16, name="ones2", tag="ones")
            nc.vector.memset(ones2[:], 1.0)

            def vcol(v, m):
                i = v * 8 + m
                return vec_t[:, i : i + 1]

            # ---- x load (bf16 copy cast on host) ----
            xbf = small.tile([128, 4, 1024], dt.bfloat16, name="xbf", tag="xbf")
            ph0 = []

            # ---- weight loads (stage A + B + prefetch) ----
            def defer(dma_inst, after, why):
                for up in after:
                    add_dep_helper(dma_inst.ins, up.ins, reason=why)
                return dma_inst

            for k in range(4):
                ph0.append(nc.sync.dma_start(xbf[:, k, :], xbf_d[:, k, :]))
            wg1 = wpool.tile([128, 4, 1024], dt.bfloat16, name="wg1", tag="w")
            for k in range(4):
                ph0.append(nc.sync.dma_start(wg1[:, k, :], wg1_d[:, k, :]))
            wb1 = wpool.tile([128, 4, 1024], dt.bfloat16, name="wb1", tag="w")
            ph0c = [defer(nc.sync.dma_start(wb1[:], wb1_d[:]), ph0, "ph0b")]
            wo = small.tile([128, 4, 256], dt.bfloat16, name="wo", tag="wo")
            ph0c.append(defer(nc.sync.dma_start(wo[:], wo_d[:]), ph0, "ph0b"))

            wg1g = wpool.tile([128, 8, 1024], dt.bfloat16, name="wg1g", tag="w")
            ph1 = [defer(nc.sync.dma_start(wg1g[:], wg1g_d[:]), ph0c, "ph1")]
            wg1be = wpool.tile([128, 8, 1024], dt.bfloat16, name="wg1be", tag="w")
            ph1.append(defer(nc.sync.dma_start(wg1be[:], wg1be_d[:]), ph0c, "ph1"))

            def conv_mm(pt, wt, m, nh, kchunks, rhs, nfree=512):
                for k in range(kchunks):
                    nc.tensor.matmul(
                        pt[:, :nfree],
                        lhsT=wt[:, k, m * 128 : (m + 1) * 128],
                        rhs=rhs[:, k, nh * nfree : (nh + 1) * nfree],
                        start=(k == 0),
                        stop=(k == kchunks - 1),
                    )

            # ---- stage A: green_1 / blue_1 ----
            green1 = acts.tile([128, 8, 1024], dt.bfloat16, name="green1", tag="a")
            for m in range(8):
                for nh in range(2):
                    pt = ps.tile([128, 512], dt.float32, name="pA", tag="ps")
                    conv_mm(pt, wg1, m, nh, 4, xbf)
                    nc.scalar.activation(
                        green1[:, m, nh * 512 : (nh + 1) * 512],
                        pt[:],
                        AF.Identity,
                        bias=vcol(V_BG1, m),
                    )
            blue1 = acts.tile([128, 8, 1024], dt.bfloat16, name="blue1", tag="a")
            for m in range(8):
                for nh in range(2):
                    pt = ps.tile([128, 512], dt.float32, name="pA2", tag="ps")
                    conv_mm(pt, wb1, m, nh, 4, xbf)
                    nc.scalar.activation(
                        blue1[:, m, nh * 512 : (nh + 1) * 512],
                        pt[:],
                        AF.Identity,
                        bias=vcol(V_BB1, m),
                    )

            # late weight loads reuse wg1/wb1 slots once stage A is done
            wb1g = wpool.tile([128, 8, 1024], dt.bfloat16, name="wb1g", tag="w")
            ph2 = [defer(nc.sync.dma_start(wb1g[:], wb1g_d[:]), ph1, "ph2")]
            wb1be = wpool.tile([128, 8, 1024], dt.bfloat16, name="wb1be", tag="w")
            ph2.append(defer(nc.sync.dma_start(wb1be[:], wb1be_d[:]), ph1, "ph2"))
            wg2 = w2.tile([128, 8, 1024], dt.bfloat16, name="wg2", tag="w2")
            defer(nc.sync.dma_start(wg2[:], wg2_d[:]), ph2, "ph3")
            wb2 = w2.tile([128, 8, 1024], dt.bfloat16, name="wb2", tag="w2")
            defer(nc.sync.dma_start(wb2[:], wb2_d[:]), ph2, "ph3")

            # ---- stage B+C interleaved per m-tile ----
            gdr = acts.tile([128, 8, 1024], dt.bfloat16, name="gdr", tag="a")
            bdr = acts.tile([128, 8, 1024], dt.bfloat16, name="bdr", tag="a")
            for m in range(8):
                g1g_t = gbp.tile([128, 1024], dt.bfloat16, name="g1g_t", tag="g1g")
                for nh in range(2):
                    pt = ps.tile([128, 512], dt.float32, name="pB", tag="ps")
                    conv_mm(pt, wg1g, m, nh, 8, green1)
                    nc.scalar.activation(
                        g1g_t[:, nh * 512 : (nh + 1) * 512],
                        pt[:],
                        AF.Identity,
                        bias=vcol(V_BG1G, m),
                    )
                g1be_t = gbp.tile([128, 1024], dt.bfloat16, name="g1be_t", tag="g1be")
                for nh in range(2):
                    pt = ps.tile([128, 512], dt.float32, name="pB2", tag="ps")
                    conv_mm(pt, wg1be, m, nh, 8, green1)
                    nc.scalar.activation(
                        g1be_t[:, nh * 512 : (nh + 1) * 512],
                        pt[:],
                        AF.Identity,
                        bias=vcol(V_BG1BE, m),
                    )
                b1g_t = gbp.tile([128, 1024], dt.bfloat16, name="b1g_t", tag="b1g")
                for nh in range(2):
                    pt = ps.tile([128, 512], dt.float32, name="pB3", tag="ps")
                    conv_mm(pt, wb1g, m, nh, 8, blue1)
                    nc.vector.tensor_scalar_add(
                        b1g_t[:, nh * 512 : (nh + 1) * 512], pt[:], vcol(V_BB1G, m)
                    )
                b1be_t = gbp.tile([128, 1024], dt.bfloat16, name="b1be_t", tag="b1be")
                for nh in range(2):
                    pt = ps.tile([128, 512], dt.float32, name="pB4", tag="ps")
                    conv_mm(pt, wb1be, m, nh, 8, blue1)
                    nc.vector.tensor_scalar_add(
                        b1be_t[:, nh * 512 : (nh + 1) * 512], pt[:], vcol(V_BB1BE, m)
                    )

                # C green: relu((s_g1*green1 + t_g1)*b1_gamma + b1_beta)
                cu = ctmp.tile([128, 1024], dt.bfloat16, name="cu", tag="cu")
                nc.vector.scalar_tensor_tensor(
                    cu[:], b1g_t[:], vcol(V_TG1, m), b1be_t[:], ALU.mult, ALU.add
                )
                cv = ctmp.tile([128, 1024], dt.bfloat16, name="cv", tag="cv")
                nc.vector.scalar_tensor_tensor(
                    cv[:], green1[:, m, :], vcol(V_SG1, m), b1g_t[:],
                    ALU.mult, ALU.mult,
                )
                cw = ctmp.tile([128, 1024], dt.bfloat16, name="cw", tag="cw")
                nc.vector.tensor_tensor(cw[:], cu[:], cv[:], ALU.add)
                nc.scalar.activation(gdr[:, m, :], cw[:], AF.Relu)

                # C blue: relu((s_b1*blue1 + t_b1)*g1_gamma + g1_beta)
                cu2 = ctmp.tile([128, 1024], dt.bfloat16, name="cu2", tag="cu")
                nc.vector.scalar_tensor_tensor(
                    cu2[:], g1g_t[:], vcol(V_TB1, m), g1be_t[:], ALU.mult, ALU.add
                )
                cv2 = ctmp.tile([128, 1024], dt.bfloat16, name="cv2", tag="cv")
                nc.vector.scalar_tensor_tensor(
                    cv2[:], blue1[:, m, :], vcol(V_SB1, m), g1g_t[:],
                    ALU.mult, ALU.mult,
                )
                cw2 = ctmp.tile([128, 1024], dt.bfloat16, name="cw2", tag="cw")
                nc.vector.tensor_tensor(cw2[:], cu2[:], cv2[:], ALU.add)
                nc.scalar.activation(bdr[:, m, :], cw2[:], AF.Relu)

            if dbg:
                nc.gpsimd.dma_start(dbg_d["green1"][:], green1[:])
                nc.gpsimd.dma_start(dbg_d["blue1"][:], blue1[:])
                nc.gpsimd.dma_start(dbg_d["gdr"][:], gdr[:])
                nc.gpsimd.dma_start(dbg_d["bdr"][:], bdr[:])

            # ---- stage E: orangeT = x^T @ WoT (fills the C->D pipeline gap) ----
            ot = acts.tile([128, 8, 256], dt.bfloat16, name="ot", tag="a")
            for nt in range(8):
                pt = ps.tile([128, 512], dt.float32, name="pE", tag="ps")
                for k in range(4):
                    nc.tensor.matmul(
                        pt[:, :256],
                        lhsT=xbf[:, k, nt * 128 : (nt + 1) * 128],
                        rhs=wo[:, k, :],
                        start=(k == 0),
                        stop=False,
                    )
                # orange bias, exact to fp32 via hi/lo rank-1 rows
                nc.tensor.matmul(
                    pt[:, :256],
                    lhsT=ones2[:, :128],
                    rhs=bo2_t[:],
                    start=False,
                    stop=True,
                )
                nc.scalar.copy(ot[:, nt, :], pt[:, :256])

            # ---- stage D: green2 / blue2 ----
            green2 = wpool.tile([128, 8, 1024], dt.bfloat16, name="green2", tag="w")
            for m in range(8):
                for nh in range(2):
                    pt = ps.tile([128, 512], dt.float32, name="pD", tag="ps")
                    conv_mm(pt, wg2, m, nh, 8, gdr)
                    nc.scalar.activation(
                        green2[:, m, nh * 512 : (nh + 1) * 512],
                        pt[:],
                        AF.Identity,
                        bias=vcol(V_BG2, m),
                    )
            blue2 = wpool.tile([128, 8, 1024], dt.bfloat16, name="blue2", tag="w")
            for m in range(8):
                for nh in range(2):
                    pt = ps.tile([128, 512], dt.float32, name="pD2", tag="ps")
                    conv_mm(pt, wb2, m, nh, 8, bdr)
                    nc.scalar.activation(
                        blue2[:, m, nh * 512 : (nh + 1) * 512],
                        pt[:],
                        AF.Identity,
                        bias=vcol(V_BB2, m),
                    )
            wd4 = w2.tile([128, 2, 512], dt.bfloat16, name="wd4", tag="w2")
            nc.sync.dma_start(wd4[:], wd4_d[:])

            if dbg:
                nc.gpsimd.dma_start(dbg_d["ot"][:], ot[:])
                nc.gpsimd.dma_start(dbg_d["green2"][:], green2[:])
                nc.gpsimd.dma_start(dbg_d["blue2"][:], blue2[:])

            # ---- stage F: davT = sigmoid(blue2^T @ green2); dav3 = of @ dav2^T ----
            d3ps = [
                ps3.tile([128, 512], dt.float32, name=f"d3ps{i}", tag=f"d3{i}")
                for i in range(4)
            ]
            for m in range(8):
                d2 = d2p.tile([128, 1024], dt.bfloat16, name="d2", tag="d2")
                for nh in range(2):
                    pt = ps.tile([128, 512], dt.float32, name="pF", tag="ps")
                    for k in range(8):
                        nc.tensor.matmul(
                            pt[:],
                            lhsT=blue2[:, k, m * 128 : (m + 1) * 128],
                            rhs=green2[:, k, nh * 512 : (nh + 1) * 512],
                            start=(k == 0),
                            stop=(k == 7),
                        )
                    nc.scalar.activation(
                        d2[:, nh * 512 : (nh + 1) * 512], pt[:], AF.Sigmoid
                    )
                if dbg:
                    nc.gpsimd.dma_start(dbg_d["d2"][m], d2[:])
                for ch in range(2):
                    for nh in range(2):
                        nc.tensor.matmul(
                            d3ps[ch * 2 + nh][:],
                            lhsT=ot[:, m, ch * 128 : (ch + 1) * 128],
                            rhs=d2[:, nh * 512 : (nh + 1) * 512],
                            start=(m == 0),
                            stop=(m == 7),
                        )
            d3sb = small.tile([128, 2, 1024], dt.bfloat16, name="d3sb", tag="xbf")
            for ch in range(2):
                nc.vector.tensor_copy(d3sb[:, ch, 0:512], d3ps[ch * 2 + 0][:])
                nc.scalar.copy(d3sb[:, ch, 512:1024], d3ps[ch * 2 + 1][:])

            if dbg:
                nc.gpsimd.dma_start(dbg_d["d3sb"][:], d3sb[:])

            # ---- stage G: dav4 + residual ----
            xf32 = acts.tile([128, 4, 1024], dt.float32, name="xf32", tag="a")
            nc.sync.dma_start(xf32[:], x_d[:])
            for o in range(4):
                for nh in range(2):
                    pt = ps.tile([128, 512], dt.float32, name="pG", tag="ps")
                    for c2 in range(2):
                        nc.tensor.matmul(
                            pt[:],
                            lhsT=wd4[:, c2, o * 128 : (o + 1) * 128],
                            rhs=d3sb[:, c2, nh * 512 : (nh + 1) * 512],
                            start=(c2 == 0),
                            stop=(c2 == 1),
                        )
                    outt = outp.tile([128, 512], dt.float32, name="outt", tag="out")
                    nc.vector.scalar_tensor_tensor(
                        outt[:],
                        pt[:],
                        vec_t[:, 96 + o : 97 + o],
                        xf32[:, o, nh * 512 : (nh + 1) * 512],
                        ALU.add,
                        ALU.add,
                    )
                    nc.gpsimd.dma_start(
                        out_d[:, o, nh * 512 : (nh + 1) * 512], outt[:]
                    )

    nc.compile()
    return nc


def _prep_inputs(x, params):
    """Host-side: transpose/cast weights, fold BN, pack per-channel vectors."""
    g = lambda k: np.asarray(params[k], dtype=F32)

    def bnfold(bn):
        w = np.asarray(bn["weight"], F32)
        b = np.asarray(bn["bias"], F32)
        mu = np.asarray(bn["mean"], F32)
        var = np.asarray(bn["var"], F32)
        s = w / np.sqrt(var + EPS)
        t = b - mu * s
        return s, t

    s_g1, t_g1 = bnfold(params["g1_bn"])
    s_b1, t_b1 = bnfold(params["b1_bn"])
    s_d4, t_d4 = bnfold(params["d4_bn"])

    wT = lambda k: np.ascontiguousarray(g(k).T).astype(BF16)
    ins = {
        "wg1t": wT("g1_w"),
        "wb1t": wT("b1_w"),
        "wg1gt": wT("g1g_w"),
        "wg1bet": wT("g1be_w"),
        "wb1gt": wT("b1g_w"),
        "wb1bet": wT("b1be_w"),
        "wg2t": wT("g2_w"),
        "wb2t": wT("b2_w"),
        "wot": wT("orange_w"),
        "wd4t": np.ascontiguousarray((g("d4_w") * s_d4[:, None]).T).astype(BF16),
    }

    bo = g("orange_b")
    bo_hi = bo.astype(BF16)
    bo_lo = (bo - bo_hi.astype(F32)).astype(BF16)
    ins["bo2"] = np.ascontiguousarray(np.stack([bo_hi, bo_lo]))

    vecs = np.zeros((128, 100), dtype=F32)
    packs = [
        g("g1_b"), s_g1, t_g1, g("b1_b"), s_b1, t_b1,
        g("g1g_b"), g("g1be_b"), g("b1g_b"), g("b1be_b"),
        g("g2_b"), g("b2_b"),
    ]
    for v, vec in enumerate(packs):
        vecs[:, v * 8 : (v + 1) * 8] = vec.reshape(8, 128).T
    b4p = g("d4_b") * s_d4 + t_d4
    vecs[:, 96:100] = b4p.reshape(4, 128).T
    ins["vecs"] = vecs

    x = np.asarray(x, dtype=F32)
    in_maps = []
    for i in range(N_CORES):
        m = dict(ins)
        m["x"] = np.ascontiguousarray(x[i])
        m["xbf16"] = np.ascontiguousarray(x[i].reshape(512, 1024).astype(BF16))
        in_maps.append(m)
    return in_maps


def kernel(x, params):
    global _compiled_nc, last_exec_time_ns, last_results
    if _compiled_nc is None:
        _compiled_nc = _build()
    nc = _compiled_nc
    in_maps = _prep_inputs(x, params)
    trace = os.environ.get("BASS_KERNEL_TRACE", "0") == "1"
    res = run_bass_kernel_spmd(nc, in_maps, list(range(N_CORES)), trace=trace)
    last_exec_time_ns = res.exec_time_ns
    last_results = res
    out = np.stack(
        [res.results[i]["out"].reshape(512, 32, 32) for i in range(N_CORES)]
    )
    return out.astype(np.float32)


# revision 22
# speedup vs baseline: 1.0147x; 1.0042x over previous
"""Trainium2 Bass kernel for nn_DepthAttention.

Data-parallel over batch: B=8 images -> 8 NeuronCores, one image per core.
Each core runs the full pipeline for its [512, 32, 32] image:
  - all 1x1 convs as tiled matmuls (bf16 inputs, fp32 PSUM accumulation)
  - BN folds into per-channel scale/shift vectors applied on-chip
  - FiLM denorm + relu on DVE/ACT
  - depth-attention (dav) computed transposed so every matmul has its
    contraction dim on partitions with no on-chip transposes
  - final conv's BN folded into the weight host-side; residual add uses
    the original fp32 x so the passthrough term is exact.
"""

import os
import sys

for _p in ("/opt/trn_rl_repo",):
    if os.path.isdir(_p) and _p not in sys.path:
        sys.path.append(_p)

import numpy as np
import ml_dtypes

import concourse.bass as bass
import concourse.tile as tile
from concourse import bacc, mybir
from concourse.bass_utils import run_bass_kernel_spmd
from concourse.tile_rust import add_dep_helper

BF16 = ml_dtypes.bfloat16
F32 = np.float32
EPS = 1e-5
N_CORES = 8
P = 128

AF = mybir.ActivationFunctionType
ALU = mybir.AluOpType
dt = mybir.dt

# Filled by the last kernel() call (test harness reads these).
last_exec_time_ns = None
last_results = None

_compiled_nc = None


def _build(dbg=False):
    """Emit the single-core Bass/Tile program (SPMD across 8 cores)."""
    nc = bacc.Bacc(
        "TRN2", target_bir_lowering=False, debug=False, num_devices=N_CORES
    )

    def dram_in(name, shape, dtype):
        return nc.dram_tensor(name, shape, dtype, kind="ExternalInput").ap()

    # DRAM views are pre-rearranged so partitions index contiguous rows.
    x_d = dram_in("x", [512, 32, 32], dt.float32).rearrange(
        "(ko p) h w -> p ko (h w)", p=P
    )
    xbf_d = dram_in("xbf16", [512, 1024], dt.bfloat16).rearrange(
        "(ko p) n -> p ko n", p=P
    )
    wg1_d = dram_in("wg1t", [512, 1024], dt.bfloat16).rearrange(
        "(ko p) c -> p ko c", p=P
    )
    wb1_d = dram_in("wb1t", [512, 1024], dt.bfloat16).rearrange(
        "(ko p) c -> p ko c", p=P
    )
    wg1g_d = dram_in("wg1gt", [1024, 1024], dt.bfloat16).rearrange(
        "(ko p) c -> p ko c", p=P
    )
    wg1be_d = dram_in("wg1bet", [1024, 1024], dt.bfloat16).rearrange(
        "(ko p) c -> p ko c", p=P
    )
    wb1g_d = dram_in("wb1gt", [1024, 1024], dt.bfloat16).rearrange(
        "(ko p) c -> p ko c", p=P
    )
    wb1be_d = dram_in("wb1bet", [1024, 1024], dt.bfloat16).rearrange(
        "(ko p) c -> p ko c", p=P
    )
    wg2_d = dram_in("wg2t", [1024, 1024], dt.bfloat16).rearrange(
        "(ko p) c -> p ko c", p=P
    )
    wb2_d = dram_in("wb2t", [1024, 1024], dt.bfloat16).rearrange(
        "(ko p) c -> p ko c", p=P
    )
    wo_d = dram_in("wot", [512, 256], dt.bfloat16).rearrange(
        "(ko p) c -> p ko c", p=P
    )
    wd4_d = dram_in("wd4t", [256, 512], dt.bfloat16).rearrange(
        "(ko p) c -> p ko c", p=P
    )
    bo2_d = dram_in("bo2", [2, 256], dt.bfloat16)
    vec_d = dram_in("vecs", [128, 100], dt.float32)
    dbg_d = {}
    if dbg:
        for nm, shp in [
            ("green1", [128, 8, 1024]), ("blue1", [128, 8, 1024]),
            ("gdr", [128, 8, 1024]), ("bdr", [128, 8, 1024]),
            ("ot", [128, 8, 256]), ("green2", [128, 8, 1024]),
            ("blue2", [128, 8, 1024]), ("d3sb", [128, 2, 1024]),
        ]:
            dbg_d[nm] = nc.dram_tensor(
                f"dbg_{nm}", shp, dt.bfloat16, kind="ExternalOutput"
            ).ap()
        dbg_d["d2"] = nc.dram_tensor(
            "dbg_d2", [8, 128, 1024], dt.bfloat16, kind="ExternalOutput"
        ).ap()
    out_d = nc.dram_tensor(
        "out", [512, 1024], dt.float32, kind="ExternalOutput"
    ).ap().rearrange("(o p) n -> p o n", p=P)

    # vec column layout (see _prep_inputs): 12 vectors x 8 m-cols, then b4'.
    V_BG1, V_SG1, V_TG1, V_BB1, V_SB1, V_TB1 = 0, 1, 2, 3, 4, 5
    V_BG1G, V_BG1BE, V_BB1G, V_BB1BE, V_BG2, V_BB2 = 6, 7, 8, 9, 10, 11

    with tile.TileContext(nc) as tc:
        from contextlib import ExitStack

        with ExitStack() as ctx:
            wpool = ctx.enter_context(tc.tile_pool(name="wpool", bufs=4))
            w2 = ctx.enter_context(tc.tile_pool(name="w2", bufs=2))
            acts = ctx.enter_context(tc.tile_pool(name="acts", bufs=4))
            gbp = ctx.enter_context(tc.tile_pool(name="gbp", bufs=2))
            small = ctx.enter_context(tc.tile_pool(name="small", bufs=1))
            ctmp = ctx.enter_context(tc.tile_pool(name="ctmp", bufs=1))
            d2p = ctx.enter_context(tc.tile_pool(name="d2p", bufs=2))
            outp = ctx.enter_context(tc.tile_pool(name="outp", bufs=2))
            ps = ctx.enter_context(tc.tile_pool(name="ps", bufs=4, space="PSUM"))
            ps3 = ctx.enter_context(tc.tile_pool(name="ps3", bufs=1, space="PSUM"))

            # ---- constant/small tiles ----
            vec_t = small.tile([128, 100], dt.float32, name="vec_t", tag="vec")
            nc.sync.dma_start(vec_t[:], vec_d[:])
            bo2_t = small.tile([2, 256], dt.bfloat16, name="bo2_t", tag="bo2")
            nc.sync.dma_start(bo2_t[:], bo2_d[:])
            ones2 = small.tile([2, 512], dt.bfloat16, name="ones2", tag="ones")
            nc.vector.memset(ones2[:], 1.0)

            def vcol(v, m):
                i = v * 8 + m
                return vec_t[:, i : i + 1]

            # ---- x load (bf16 copy cast on host) ----
            xbf = small.tile([128, 4, 1024], dt.bfloat16, name="xbf", tag="xbf")
            ph0 = []

            # ---- weight loads (stage A + B + prefetch) ----
            def defer(dma_inst, after, why):
                for up in after:
                    add_dep_helper(dma_inst.ins, up.ins, reason=why)
                return dma_inst

            for k in range(4):
                ph0.append(nc.sync.dma_start(xbf[:, k, :], xbf_d[:, k, :]))
            wg1 = wpool.tile([128, 4, 1024], dt.bfloat16, name="wg1", tag="w")
            for k in range(4):
                ph0.append(nc.sync.dma_start(wg1[:, k, :], wg1_d[:, k, :]))
            wb1 = wpool.tile([128, 4, 1024], dt.bfloat16, name="wb1", tag="w")
            ph0c = [defer(nc.sync.dma_start(wb1[:], wb1_d[:]), ph0, "ph0b")]
            wo = small.tile([128, 4, 256], dt.bfloat16, name="wo", tag="wo")
            ph0c.append(defer(nc.sync.dma_start(wo[:], wo_d[:]), ph0, "ph0b"))

            wg1g = wpool.tile([128, 8, 1024], dt.bfloat16, name="wg1g", tag="w")
            ph1 = [defer(nc.sync.dma_start(wg1g[:], wg1g_d[:]), ph0c, "ph1")]
            wg1be = wpool.tile([128, 8, 1024], dt.bfloat16, name="wg1be", tag="w")
            ph1.append(defer(nc.sync.dma_start(wg1be[:], wg1be_d[:]), ph0c, "ph1"))

            def conv_mm(pt, wt, m, nh, kchunks, rhs, nfree=512):
                for k in range(kchunks):
                    nc.tensor.matmul(
                        pt[:, :nfree],
                        lhsT=wt[:, k, m * 128 : (m + 1) * 128],
                        rhs=rhs[:, k, nh * nfree : (nh + 1) * nfree],
                        start=(k == 0),
                        stop=(k == kchunks - 1),
                    )

            # ---- stage A: green_1 / blue_1 ----
            green1 = acts.tile([128, 8, 1024], dt.bfloat16, name="green1", tag="a")
            for m in range(8):
                for nh in range(2):
                    pt = ps.tile([128, 512], dt.float32, name="pA", tag="ps")
                    conv_mm(pt, wg1, m, nh, 4, xbf)
                    nc.scalar.activation(
                        green1[:, m, nh * 512 : (nh + 1) * 512],
                        pt[:],
                        AF.Identity,
                        bias=vcol(V_BG1, m),
                    )
            blue1 = acts.tile([128, 8, 1024], dt.bfloat16, name="blue1", tag="a")
            for m in range(8):
                for nh in range(2):
                    pt = ps.tile([128, 512], dt.float32, name="pA2", tag="ps")
                    conv_mm(pt, wb1, m, nh, 4, xbf)
                    nc.scalar.activation(
                        blue1[:, m, nh * 512 : (nh + 1) * 512],
                        pt[:],
                        AF.Identity,
                        bias=vcol(V_BB1, m),
                    )

            # late weight loads reuse wg1/wb1 slots once stage A is done
            wb1g = wpool.tile([128, 8, 1024], dt.bfloat16, name="wb1g", tag="w")
            ph2 = [defer(nc.sync.dma_start(wb1g[:], wb1g_d[:]), ph1, "ph2")]
            wb1be = wpool.tile([128, 8, 1024], dt.bfloat16, name="wb1be", tag="w")
            ph2.append(defer(nc.sync.dma_start(wb1be[:], wb1be_d[:]), ph1, "ph2"))
            wg2 = w2.tile([128, 8, 1024], dt.bfloat16, name="wg2", tag="w2")
            defer(nc.sync.dma_start(wg2[:], wg2_d[:]), ph2, "ph3")
            wb2 = w2.tile([128, 8, 1024], dt.bfloat16, name="wb2", tag="w2")
            defer(nc.sync.dma_start(wb2[:], wb2_d[:]), ph2, "ph3")

            # ---- stage B+C interleaved per m-tile ----
            gdr = acts.tile([128, 8, 1024], dt.bfloat16, name="gdr", tag="a")
            bdr = acts.tile([128, 8, 1024], dt.bfloat16, name="bdr", tag="a")
            for m in range(8):
                g1g_t = gbp.tile([128, 1024], dt.bfloat16, name="g1g_t", tag="g1g")
                for nh in range(2):
                    pt = ps.tile([128, 512], dt.float32, name="pB", tag="ps")
                    conv_mm(pt, wg1g, m, nh, 8, green1)
                    nc.scalar.activation(
                        g1g_t[:, nh * 512 : (nh + 1) * 512],
                        pt[:],
                        AF.Identity,
                        bias=vcol(V_BG1G, m),
                    )
                g1be_t = gbp.tile([128, 1024], dt.bfloat16, name="g1be_t", tag="g1be")
                for nh in range(2):
                    pt = ps.tile([128, 512], dt.float32, name="pB2", tag="ps")
                    conv_mm(pt, wg1be, m, nh, 8, green1)
                    nc.scalar.activation(
                        g1be_t[:, nh * 512 : (nh + 1) * 512],
                        pt[:],
                        AF.Identity,
                        bias=vcol(V_BG1BE, m),
                    )
                b1g_t = gbp.tile([128, 1024], dt.bfloat16, name="b1g_t", tag="b1g")
                for nh in range(2):
                    pt = ps.tile([128, 512], dt.float32, name="pB3", tag="ps")
                    conv_mm(pt, wb1g, m, nh, 8, blue1)
                    nc.vector.tensor_scalar_add(
                        b1g_t[:, nh * 512 : (nh + 1) * 512], pt[:], vcol(V_BB1G, m)
                    )
                b1be_t = gbp.tile([128, 1024], dt.bfloat16, name="b1be_t", tag="b1be")
                for nh in range(2):
                    pt = ps.tile([128, 512], dt.float32, name="pB4", tag="ps")
                    conv_mm(pt, wb1be, m, nh, 8, blue1)
                    nc.vector.tensor_scalar_add(
                        b1be_t[:, nh * 512 : (nh + 1) * 512], pt[:], vcol(V_BB1BE, m)
                    )

                # C green: relu((s_g1*green1 + t_g1)*b1_gamma + b1_beta)
                cu = ctmp.tile([128, 1024], dt.bfloat16, name="cu", tag="cu")
                nc.vector.scalar_tensor_tensor(
                    cu[:], b1g_t[:], vcol(V_TG1, m), b1be_t[:], ALU.mult, ALU.add
                )
                cv = ctmp.tile([128, 1024], dt.bfloat16, name="cv", tag="cv")
                nc.vector.scalar_tensor_tensor(
                    cv[:], green1[:, m, :], vcol(V_SG1, m), b1g_t[:],
                    ALU.mult, ALU.mult,
                )
                cw = ctmp.tile([128, 1024], dt.bfloat16, name="cw", tag="cw")
                nc.vector.tensor_tensor(cw[:], cu[:], cv[:], ALU.add)
                nc.scalar.activation(gdr[:, m, :], cw[:], AF.Relu)

                # C blue: relu((s_b1*blue1 + t_b1)*g1_gamma + g1_beta)
                cu2 = ctmp.tile([128, 1024], dt.bfloat16, name="cu2", tag="cu")
                nc.vector.scalar_tensor_tensor(
                    cu2[:], g1g_t[:], vcol(V_TB1, m), g1be_t[:], ALU.mult, ALU.add
                )
                cv2 = ctmp.tile([128, 1024], dt.bfloat16, name="cv2", tag="cv")
                nc.vector.scalar_tensor_tensor(
                    cv2[:], blue1[:, m, :], vcol(V_SB1, m), g1g_t[:],
                    ALU.mult, ALU.mult,
                )
                cw2 = ctmp.tile([128, 1024], dt.bfloat16, name="cw2", tag="cw")
                nc.vector.tensor_tensor(cw2[:], cu2[:], cv2[:], ALU.add)
                nc.scalar.activation(bdr[:, m, :], cw2[:], AF.Relu)

            if dbg:
                nc.gpsimd.dma_start(dbg_d["green1"][:], green1[:])
                nc.gpsimd.dma_start(dbg_d["blue1"][:], blue1[:])
                nc.gpsimd.dma_start(dbg_d["gdr"][:], gdr[:])
                nc.gpsimd.dma_start(dbg_d["bdr"][:], bdr[:])

            # ---- stage E: orangeT = x^T @ WoT (fills the C->D pipeline gap) ----
            ot = acts.tile([128, 8, 256], dt.bfloat16, name="ot", tag="a")
            for nt in range(8):
                pt = ps.tile([128, 512], dt.float32, name="pE", tag="ps")
                for k in range(4):
                    nc.tensor.matmul(
                        pt[:, :256],
                        lhsT=xbf[:, k, nt * 128 : (nt + 1) * 128],
                        rhs=wo[:, k, :],
                        start=(k == 0),
                        stop=False,
                    )
                # orange bias, exact to fp32 via hi/lo rank-1 rows
                nc.tensor.matmul(
                    pt[:, :256],
                    lhsT=ones2[:, :128],
                    rhs=bo2_t[:],
                    start=False,
                    stop=True,
                )
                nc.scalar.copy(ot[:, nt, :], pt[:, :256])

            # ---- stage D: green2 / blue2 ----
            green2 = wpool.tile([128, 8, 1024], dt.bfloat16, name="green2", tag="w")
            for m in range(8):
                for nh in range(2):
                    pt = ps.tile([128, 512], dt.float32, name="pD", tag="ps")
                    conv_mm(pt, wg2, m, nh, 8, gdr)
                    nc.scalar.activation(
                        green2[:, m, nh * 512 : (nh + 1) * 512],
                        pt[:],
                        AF.Identity,
                        bias=vcol(V_BG2, m),
                    )
            blue2 = wpool.tile([128, 8, 1024], dt.bfloat16, name="blue2", tag="w")
            for m in range(8):
                for nh in range(2):
                    pt = ps.tile([128, 512], dt.float32, name="pD2", tag="ps")
                    conv_mm(pt, wb2, m, nh, 8, bdr)
                    nc.scalar.activation(
                        blue2[:, m, nh * 512 : (nh + 1) * 512],
                        pt[:],
                        AF.Identity,
                        bias=vcol(V_BB2, m),
                    )
            wd4 = w2.tile([128, 2, 512], dt.bfloat16, name="wd4", tag="w2")
            nc.sync.dma_start(wd4[:], wd4_d[:])

            if dbg:
                nc.gpsimd.dma_start(dbg_d["ot"][:], ot[:])
                nc.gpsimd.dma_start(dbg_d["green2"][:], green2[:])
                nc.gpsimd.dma_start(dbg_d["blue2"][:], blue2[:])

            # ---- stage F: davT = sigmoid(blue2^T @ green2); dav3 = of @ dav2^T ----
            d3ps = [
                ps3.tile([128, 512], dt.float32, name=f"d3ps{i}", tag=f"d3{i}")
                for i in range(4)
            ]
            for m in range(8):
                d2 = d2p.tile([128, 1024], dt.bfloat16, name="d2", tag="d2")
                for nh in range(2):
                    pt = ps.tile([128, 512], dt.float32, name="pF", tag="ps")
                    for k in range(8):
                        nc.tensor.matmul(
                            pt[:],
                            lhsT=blue2[:, k, m * 128 : (m + 1) * 128],
                            rhs=green2[:, k, nh * 512 : (nh + 1) * 512],
                            start=(k == 0),
                            stop=(k == 7),
                        )
                    nc.scalar.activation(
                        d2[:, nh * 512 : (nh + 1) * 512], pt[:], AF.Sigmoid
                    )
                if dbg:
                    nc.gpsimd.dma_start(dbg_d["d2"][m], d2[:])
                for ch in range(2):
                    for nh in range(2):
                        nc.tensor.matmul(
                            d3ps[ch * 2 + nh][:],
                            lhsT=ot[:, m, ch * 128 : (ch + 1) * 128],
                            rhs=d2[:, nh * 512 : (nh + 1) * 512],
                            start=(m == 0),
                            stop=(m == 7),
                        )
            d3sb = small.tile([128, 2, 1024], dt.bfloat16, name="d3sb", tag="xbf")
            for ch in range(2):
                nc.vector.tensor_copy(d3sb[:, ch, 0:512], d3ps[ch * 2 + 0][:])
                nc.scalar.copy(d3sb[:, ch, 512:1024], d3ps[ch * 2 + 1][:])

            if dbg:
                nc.gpsimd.dma_start(dbg_d["d3sb"][:], d3sb[:])

            # ---- stage G: dav4 + residual ----
            xf32 = acts.tile([128, 4, 1024], dt.float32, name="xf32", tag="a")
            nc.sync.dma_start(xf32[:], x_d[:])
            for o in range(4):
                for nh in range(2):
                    pt = ps.tile([128, 512], dt.float32, name="pG", tag="ps")
                    for c2 in range(2):
                        nc.tensor.matmul(
                            pt[:],
                            lhsT=wd4[:, c2, o * 128 : (o + 1) * 128],
                            rhs=d3sb[:, c2, nh * 512 : (nh + 1) * 512],
                            start=(c2 == 0),
                            stop=(c2 == 1),
                        )
                    outt = outp.tile([128, 512], dt.float32, name="outt", tag="out")
                    nc.vector.scalar_tensor_tensor(
                        outt[:],
                        pt[:],
                        vec_t[:, 96 + o : 97 + o],
                        xf32[:, o, nh * 512 : (nh + 1) * 512],
                        ALU.add,
                        ALU.add,
                    )
                    nc.gpsimd.dma_start(
                        out_d[:, o, nh * 512 : (nh + 1) * 512], outt[:]
                    )

    nc.compile()
    return nc


def _prep_inputs(x, params):
    """Host-side: transpose/cast weights, fold BN, pack per-channel vectors."""
    g = lambda k: np.asarray(params[k], dtype=F32)

    def bnfold(bn):
        w = np.asarray(bn["weight"], F32)
        b = np.asarray(bn["bias"], F32)
        mu = np.asarray(bn["mean"], F32)
        var = np.asarray(bn["var"], F32)
        s = w / np.sqrt(var + EPS)
        t = b - mu * s
        return s, t

    s_g1, t_g1 = bnfold(params["g1_bn"])
    s_b1, t_b1 = bnfold(params["b1_bn"])
    s_d4, t_d4 = bnfold(params["d4_bn"])

    wT = lambda k: np.ascontiguousarray(g(k).T).astype(BF16)
    ins = {
        "wg1t": wT("g1_w"),
        "wb1t": wT("b1_w"),
        "wg1gt": wT("g1g_w"),
        "wg1bet": wT("g1be_w"),
        "wb1gt": wT("b1g_w"),
        "wb1bet": wT("b1be_w"),
        "wg2t": wT("g2_w"),
        "wb2t": wT("b2_w"),
        "wot": wT("orange_w"),
        "wd4t": np.ascontiguousarray((g("d4_w") * s_d4[:, None]).T).astype(BF16),
    }

    bo = g("orange_b")
    bo_hi = bo.astype(BF16)
    bo_lo = (bo - bo_hi.astype(F32)).astype(BF16)
    ins["bo2"] = np.ascontiguousarray(np.stack([bo_hi, bo_lo]))

    vecs = np.zeros((128, 100), dtype=F32)
    packs = [
        g("g1_b"), s_g1, t_g1, g("b1_b"), s_b1, t_b1,
        g("g1g_b"), g("g1be_b"), g("b1g_b"), g("b1be_b"),
        g("g2_b"), g("b2_b"),
    ]
    for v, vec in enumerate(packs):
        vecs[:, v * 8 : (v + 1) * 8] = vec.reshape(8, 128).T
    b4p = g("d4_b") * s_d4 + t_d4
    vecs[:, 96:100] = b4p.reshape(4, 128).T
    ins["vecs"] = vecs

    x = np.asarray(x, dtype=F32)
    in_maps = []
    for i in range(N_CORES):
        m = dict(ins)
        m["x"] = np.ascontiguousarray(x[i])
        m["xbf16"] = np.ascontiguousarray(x[i].reshape(512, 1024).astype(BF16))
        in_maps.append(m)
    return in_maps


def kernel(x, params):
    global _compiled_nc, last_exec_time_ns, last_results
    if _compiled_nc is None:
        _compiled_nc = _build()
    nc = _compiled_nc
    in_maps = _prep_inputs(x, params)
    trace = os.environ.get("BASS_KERNEL_TRACE", "0") == "1"
    res = run_bass_kernel_spmd(nc, in_maps, list(range(N_CORES)), trace=trace)
    last_exec_time_ns = res.exec_time_ns
    last_results = res
    out = np.stack(
        [res.results[i]["out"].reshape(512, 32, 32) for i in range(N_CORES)]
    )
    return out.astype(np.float32)
